# revision 8
# baseline (speedup 1.0000x reference)
"""CenterLoss kernel for 8 Trainium2 NeuronCores (Bass/Tile).

Reference computation:
    label = argmax(predicts, axis=-1)            # [N], N = 32*256 = 8192
    d_n   = ||features_n - centers[label_n]||^2  # [N]
    loss  = (sum_n clip(d_n, EPS, INF) + N*(C-1)*EPS) / N

(The N*(C-1)*EPS term comes from the reference clipping the zeroed
mask-complement entries of the [N, C] masked distance matrix to EPS.)

Sharding: data-parallel over the flattened N axis — 1024 rows per core,
centers replicated. Per core the kernel streams its [1024, 6625] predicts
shard through SBUF in 8 [128, 6625] tiles split across both HWDGE rings.

Steady tiles (0-6) use a hybrid-precision hierarchical argmax: the ring-1
half (chunks 0-25) is cast fp32->fp16 on the otherwise-idle Activation
engine and max-reduced on DVE at the 2x 2-byte rate; the ring-2 half
(chunks 26-52) is max-reduced directly in fp32 on DVE. The combined
per-chunk maxima (fp16) feed Max8/FindIndex8 for the winning chunk, an
SWDGE regather of that chunk, a second FindIndex8 for the within-chunk
position, and an SWDGE centers-row gather. fp16 rounding is monotone, so
this computes the exact argmax of the fp16-rounded row; only rows whose
top-2 values collide within one fp16 ulp can differ from the fp32 argmax
(~1% of rows, ~1e-4 relative effect on the loss — far inside tolerance).

The last tile runs a pure-fp32 path with fine-grained column pieces (the
final piece is only 3 chunks) so the post-stream serial chain is short;
its distance is computed on DVE (tensor_tensor + tensor_tensor_reduce) to
avoid cross-engine hops in the tail. Phase-2 work is issued in stages
(A: chunk argmax + regather, B: within-chunk argmax + centers gather,
C: distance + output) interleaved across tiles so no engine head-of-line
blocks on a DMA round trip. The host sums the 8 per-core partial vectors
(the scalar "all-reduce") and applies the EPS correction.
"""

import numpy as np

import concourse.bacc as bacc
import concourse.bass as bass
import concourse.mybir as mybir
from concourse import tile
from concourse.bass_utils import run_bass_kernel_spmd

B, T, D, C = 32, 256, 96, 6625
N = B * T                  # 8192 rows total
NCORES = 8
NS = N // NCORES           # 1024 rows per core
P = 128                    # SBUF partitions
NT = NS // P               # 8 predicts tiles per core
NCH = 53                   # chunks per row for hierarchical argmax
CW = 125                   # chunk width (53 * 125 = 6625)
KH = 26                    # fp16-cast chunks per steady tile (ring-1 half)
EPS = 1e-7

# test.py toggles these module-level knobs; the grading harness just calls
# kernel(**inputs) and gets the defaults.
TRACE = False
TRACE_KWARGS = {}
LAST_RESULTS = None

# iotas[p, t] = (t*P + p) * NCH — base chunk index per (partition, tile)
_IOTAS = np.ascontiguousarray(
    ((np.arange(NT)[None, :] * P + np.arange(P)[:, None]) * NCH).astype(np.int32)
)


def _build():
    nc = bacc.Bacc("TRN2", num_devices=NCORES)
    f32 = mybir.dt.float32
    f16 = mybir.dt.float16
    u32 = mybir.dt.uint32
    pred = nc.dram_tensor("predicts", [NS, C], f32, kind="ExternalInput").ap()
    # features arrive host-pre-transposed to [P, NT*D] (partition-major) so the
    # load is one contiguous 3KB-per-partition DMA
    feat = nc.dram_tensor("features", [P, NT * D], f32, kind="ExternalInput").ap()
    # host-computed base chunk indices: iotas[p, t] = (t*P + p) * NCH
    iot = nc.dram_tensor("iotas", [P, NT], mybir.dt.int32, kind="ExternalInput").ap()
    cent = nc.dram_tensor("centers", [C, D], f32, kind="ExternalInput").ap()
    dist = nc.dram_tensor("dists", [P, NT], f32, kind="ExternalOutput").ap()
    labs = nc.dram_tensor("labels", [P, NT], u32, kind="ExternalOutput").ap()

    # flat chunk view for the winning-chunk regather: row r, chunk k lives at
    # predflat[r * NCH + k, :]
    predflat = pred.rearrange("n (k q) -> (n k) q", q=CW)

    LT = NT - 1         # last tile index (pure-fp32 fine-piece path)
    H16 = KH * CW       # 3250 fp16-side columns on steady tiles

    with tile.TileContext(nc) as tc:
        with (
            tc.tile_pool(name="pred", bufs=5) as pp,
            tc.tile_pool(name="p16", bufs=3) as hp,
            tc.tile_pool(name="small", bufs=4) as sp,
            tc.tile_pool(name="persist", bufs=1) as ps,
        ):
            ftile = ps.tile([P, NT, D], f32)
            nc.gpsimd.dma_start(ftile[:], feat.rearrange("p (t d) -> p t d", d=D))
            iotas = ps.tile([P, NT], mybir.dt.int32)
            nc.gpsimd.dma_start(iotas[:], iot[:])

            labt = ps.tile([P, NT], u32)
            ctile = ps.tile([P, NT, D], f32)
            gath = ps.tile([P, NT, CW], f32)
            gath16 = ps.tile([P, NT, CW], f16)
            offs = ps.tile([P, NT], u32)
            diff = ps.tile([P, NT, D], f32)
            sq = ps.tile([P, NT, D], f32)
            d2 = ps.tile([P, NT], f32)
            # per-chunk maxima: steady tiles in fp16, last tile in fp32
            cm16 = ps.tile([P, NT, NCH], f16)
            cm32L = ps.tile([P, NCH], f32)
            top16 = ps.tile([P, NT, 8], f16)
            top32L = ps.tile([P, 8], f32)
            cidx8s = ps.tile([P, NT, 8], u32)

            def stage_a(t):
                """chunk argmax + winning-chunk regather (needs cm[t])."""
                if t == LT:
                    nc.vector.max(out=top32L[:], in_=cm32L[:])
                    nc.vector.max_index(
                        out=cidx8s[:, t, :], in_max=top32L[:], in_values=cm32L[:]
                    )
                else:
                    nc.vector.max(out=top16[:, t, :], in_=cm16[:, t, :])
                    nc.vector.max_index(
                        out=cidx8s[:, t, :], in_max=top16[:, t, :],
                        in_values=cm16[:, t, :],
                    )
                nc.vector.tensor_add(
                    offs[:, t : t + 1], iotas[:, t : t + 1], cidx8s[:, t, 0:1]
                )
                nc.gpsimd.indirect_dma_start(
                    out=gath[:, t, :],
                    out_offset=None,
                    in_=predflat,
                    in_offset=bass.IndirectOffsetOnAxis(ap=offs[:, t : t + 1], axis=0),
                )

            def stage_b(t):
                """within-chunk argmax + centers gather (needs gath[t])."""
                widx8 = sp.tile([P, 8], u32, tag="widx8")
                if t == LT:
                    nc.vector.max_index(
                        out=widx8[:], in_max=top32L[:], in_values=gath[:, t, :]
                    )
                else:
                    nc.scalar.copy(gath16[:, t, :], gath[:, t, :])
                    nc.vector.max_index(
                        out=widx8[:], in_max=top16[:, t, :], in_values=gath16[:, t, :]
                    )
                # label = cidx * CW + widx
                nc.vector.tensor_scalar(
                    labt[:, t : t + 1], cidx8s[:, t, 0:1], float(CW), None,
                    op0=mybir.AluOpType.mult,
                )
                nc.vector.tensor_add(
                    labt[:, t : t + 1], labt[:, t : t + 1], widx8[:, 0:1]
                )
                nc.gpsimd.indirect_dma_start(
                    out=ctile[:, t, :],
                    out_offset=None,
                    in_=cent[:],
                    in_offset=bass.IndirectOffsetOnAxis(ap=labt[:, t : t + 1], axis=0),
                )

            def stage_c(t):
                """distance + dist output (needs ctile[t])."""
                if t == LT:
                    # DVE-local distance: avoids GpSimd/Act hops in the tail
                    nc.vector.tensor_tensor(
                        diff[:, t, :], ftile[:, t, :], ctile[:, t, :],
                        op=mybir.AluOpType.subtract,
                    )
                    nc.vector.tensor_tensor(
                        sq[:, t, :], diff[:, t, :], diff[:, t, :],
                        op=mybir.AluOpType.mult,
                    )
                    nc.vector.tensor_reduce(
                        d2[:, t : t + 1], sq[:, t, :],
                        axis=mybir.AxisListType.X, op=mybir.AluOpType.add,
                    )
                else:
                    nc.gpsimd.tensor_tensor(
                        diff[:, t, :], ftile[:, t, :], ctile[:, t, :],
                        op=mybir.AluOpType.subtract,
                    )
                    nc.scalar.activation(
                        sq[:, t, :], diff[:, t, :],
                        mybir.ActivationFunctionType.Square,
                        accum_out=d2[:, t : t + 1],
                    )
                # outputs leave via the GpSimd queue: a not-yet-ready write
                # parked on a HWDGE ring would head-of-line block the
                # predicts stream behind it
                nc.gpsimd.dma_start(labs[:, t : t + 1], labt[:, t : t + 1])
                nc.gpsimd.dma_start(dist[:, t : t + 1], d2[:, t : t + 1])

            for t in range(NT):
                rows = pred[t * P : (t + 1) * P, :]
                pt = pp.tile([P, C], f32, tag="pt")
                if t < LT:
                    # ring 1 (Sync HWDGE): fp16-side half; ring 2 (Act HWDGE):
                    # fp32-side half — both land over the full tile period
                    nc.sync.dma_start(pt[:, :H16], rows[:, :H16])
                    nc.scalar.dma_start(pt[:, H16:], rows[:, H16:])
                    p16 = hp.tile([P, H16], f16, tag="p16")
                    nc.scalar.copy(p16[:], pt[:, :H16])
                    # DVE order: fp32 half first (ready as soon as its DMA
                    # lands), fp16 half after the Act-engine cast completes
                    cm32 = sp.tile([P, NCH - KH], f32, tag="cm32")
                    nc.vector.reduce_max(
                        cm32[:],
                        pt[:, H16:].rearrange("p (k q) -> p k q", q=CW),
                        axis=mybir.AxisListType.X,
                    )
                    nc.vector.tensor_copy(cm16[:, t, KH:NCH], cm32[:])
                    nc.vector.reduce_max(
                        cm16[:, t, 0:KH],
                        p16[:].rearrange("p (k q) -> p k q", q=CW),
                        axis=mybir.AxisListType.X,
                    )
                else:
                    # last tile: pure fp32, fine pieces, tiny final piece so
                    # the post-stream serial chain is minimal
                    # ring totals incl. steady tiles: sync 7*26+30=212,
                    # scalar 7*27+23=212 — both rings converge at stream end
                    # with the tiny 4-chunk piece landing last
                    bounds = [(0, 13, nc.sync), (13, 25, nc.scalar),
                              (25, 38, nc.sync), (38, 49, nc.scalar),
                              (49, 53, nc.sync)]
                    for lo, hi, eng in bounds:
                        eng.dma_start(pt[:, lo * CW : hi * CW],
                                      rows[:, lo * CW : hi * CW])
                        nc.vector.reduce_max(
                            cm32L[:, lo:hi],
                            pt[:, lo * CW : hi * CW].rearrange(
                                "p (k q) -> p k q", q=CW
                            ),
                            axis=mybir.AxisListType.X,
                        )
                # staged software pipeline: A(t) | B(t-1) | C(t-2) — each
                # stage's DMA round trip completes while later tiles stream
                stage_a(t)
                if t >= 1:
                    stage_b(t - 1)
                if t >= 2:
                    stage_c(t - 2)

            # epilogue: B(LT) first so the last tile's centers gather is
            # never head-of-line blocked behind C(LT-1) work
            stage_b(LT)
            stage_c(LT - 1)
            stage_c(LT)
    nc.compile()
    return nc


def kernel(features, predicts, centers):
    global LAST_RESULTS
    feats = np.ascontiguousarray(np.asarray(features).reshape(N, D), dtype=np.float32)
    preds = np.ascontiguousarray(np.asarray(predicts).reshape(N, C), dtype=np.float32)
    cents = np.ascontiguousarray(np.asarray(centers), dtype=np.float32)

    nc = _build()
    in_maps = []
    for i in range(NCORES):
        fshard = feats[i * NS : (i + 1) * NS]  # [1024, 96]
        # [P, NT*D] partition-major layout: row t*128+p -> [p, t*D:(t+1)*D]
        fT = np.ascontiguousarray(
            fshard.reshape(NT, P, D).transpose(1, 0, 2).reshape(P, NT * D)
        )
        in_maps.append(
            {
                "predicts": preds[i * NS : (i + 1) * NS],
                "features": fT,
                "centers": cents,
                "iotas": _IOTAS,
            }
        )
    res = run_bass_kernel_spmd(
        nc, in_maps, core_ids=list(range(NCORES)), trace=TRACE, **TRACE_KWARGS
    )
    LAST_RESULTS = res

    total = 0.0
    for r in res.results:
        # EPS clip of the per-row distances happens here as part of the
        # unshard-reduce (d2 >= 0 always; only the lower clip can bind)
        total += float(np.maximum(r["dists"], EPS).astype(np.float64).sum())
    total += float(N) * (C - 1) * EPS
    return np.asarray(total / N, dtype=np.float32)


# revision 10
# speedup vs baseline: 1.0523x; 1.0523x over previous
"""CenterLoss kernel for 8 Trainium2 NeuronCores (Bass/Tile).

Reference computation:
    label = argmax(predicts, axis=-1)            # [N], N = 32*256 = 8192
    d_n   = ||features_n - centers[label_n]||^2  # [N]
    loss  = (sum_n clip(d_n, EPS, INF) + N*(C-1)*EPS) / N

(The N*(C-1)*EPS term comes from the reference clipping the zeroed
mask-complement entries of the [N, C] masked distance matrix to EPS.)

Sharding: data-parallel over the flattened N axis — 1024 rows per core,
centers replicated. Per core the kernel streams its [1024, 6625] predicts
shard through SBUF in 8 [128, 6625] tiles, split across both HWDGE rings.
All stream DMAs are enqueued up front so each ring holds its whole
schedule and is paced only by tile-pool WAR semaphores — a ring never
starves because an engine was busy computing when the next issue was due.

Per tile DVE computes per-chunk maxima (53 chunks x 125), Max8 +
FindIndex8 pick the winning chunk, an SWDGE regather fetches that chunk,
a second FindIndex8 gives the within-chunk position, and an SWDGE gather
fetches the centers row. Label arithmetic and the f-c subtract run on
GpSimd, the square+row-sum on the Activation engine, so DVE carries only
the reduce/argmax ops (it is the throughput-critical engine, ~8.1us of
work against the 8.2us DMA period). Phase-2 work is issued in stages
(A: chunk argmax + regather, B: within-chunk argmax + centers gather,
C: distance + outputs) at tile lags 0/1/2 so every DMA round trip hides
under later tiles' streaming. Outputs leave via the GpSimd queue — a
not-yet-ready write parked on a HWDGE ring would head-of-line block the
predicts stream behind it.

The last tile streams in five fine-grained pieces (the final one only 4
chunks) and computes its distance DVE-locally, so the post-stream serial
chain is just: tiny reduce -> Max8/FindIndex8 -> regather -> FindIndex8
-> centers gather -> diff/square/sum -> 512B result write. The host sums
the 8 per-core partial vectors (the scalar "all-reduce") and applies the
EPS correction.
"""

import numpy as np

import concourse.bacc as bacc
import concourse.bass as bass
import concourse.mybir as mybir
from concourse import tile
from concourse.bass_utils import run_bass_kernel_spmd

B, T, D, C = 32, 256, 96, 6625
N = B * T                  # 8192 rows total
NCORES = 8
NS = N // NCORES           # 1024 rows per core
P = 128                    # SBUF partitions
NT = NS // P               # 8 predicts tiles per core
NCH = 53                   # chunks per row for hierarchical argmax
CW = 125                   # chunk width (53 * 125 = 6625)
H1 = 26 * CW               # ring-1 half: chunks 0-25; ring-2: 26-52
EPS = 1e-7

# test.py toggles these module-level knobs; the grading harness just calls
# kernel(**inputs) and gets the defaults.
TRACE = False
TRACE_KWARGS = {}
LAST_RESULTS = None

# iotas[p, t] = (t*P + p) * NCH — base chunk index per (partition, tile)
_IOTAS = np.ascontiguousarray(
    ((np.arange(NT)[None, :] * P + np.arange(P)[:, None]) * NCH).astype(np.uint32)
)


def _build():
    nc = bacc.Bacc("TRN2", num_devices=NCORES)
    f32 = mybir.dt.float32
    u32 = mybir.dt.uint32
    pred = nc.dram_tensor("predicts", [NS, C], f32, kind="ExternalInput").ap()
    # features arrive host-pre-transposed to [P, NT*D] (partition-major) so the
    # load is one contiguous 3KB-per-partition DMA
    feat = nc.dram_tensor("features", [P, NT * D], f32, kind="ExternalInput").ap()
    # host-computed base chunk indices: iotas[p, t] = (t*P + p) * NCH
    iot = nc.dram_tensor("iotas", [P, NT], u32, kind="ExternalInput").ap()
    cent = nc.dram_tensor("centers", [C, D], f32, kind="ExternalInput").ap()
    dist = nc.dram_tensor("dists", [P, NT], f32, kind="ExternalOutput").ap()
    labs = nc.dram_tensor("labels", [P, NT], u32, kind="ExternalOutput").ap()

    # flat chunk view for the winning-chunk regather: row r, chunk k lives at
    # predflat[r * NCH + k, :]
    predflat = pred.rearrange("n (k q) -> (n k) q", q=CW)

    LT = NT - 1         # last tile index (fine-piece path)
    # last-tile pieces; ring totals incl. steady tiles: sync 7*26+30=212,
    # scalar 7*27+23=212 — the rings converge at stream end with the tiny
    # 4-chunk piece landing last
    LT_BOUNDS = [(0, 13, "sync"), (13, 25, "scalar"), (25, 38, "sync"),
                 (38, 49, "scalar"), (49, 53, "sync")]

    with tile.TileContext(nc) as tc:
        with (
            tc.tile_pool(name="pred", bufs=6) as pp,
            tc.tile_pool(name="small", bufs=4) as sp,
            tc.tile_pool(name="persist", bufs=1) as ps,
        ):
            ftile = ps.tile([P, NT, D], f32)
            nc.gpsimd.dma_start(ftile[:], feat.rearrange("p (t d) -> p t d", d=D))
            iotas = ps.tile([P, NT], u32)
            nc.gpsimd.dma_start(iotas[:], iot[:])

            labt = ps.tile([P, NT], u32)
            ctile = ps.tile([P, NT, D], f32)
            gath = ps.tile([P, NT, CW], f32)
            offs = ps.tile([P, NT], u32)
            diff = ps.tile([P, NT, D], f32)
            sq = ps.tile([P, NT, D], f32)
            d2 = ps.tile([P, NT], f32)
            cm = ps.tile([P, NT, NCH], f32)
            top8 = ps.tile([P, NT, 8], f32)
            cidx8s = ps.tile([P, NT, 8], u32)

            # ---- enqueue the whole predicts stream on both HWDGE rings ----
            pts = []
            for t in range(NT):
                rows = pred[t * P : (t + 1) * P, :]
                pt = pp.tile([P, C], f32, tag="pt")
                pts.append(pt)
                if t < LT:
                    nc.sync.dma_start(pt[:, :H1], rows[:, :H1])
                    nc.scalar.dma_start(pt[:, H1:], rows[:, H1:])
                else:
                    for lo, hi, eng in LT_BOUNDS:
                        getattr(nc, eng).dma_start(
                            pt[:, lo * CW : hi * CW], rows[:, lo * CW : hi * CW]
                        )

            def stage_a(t):
                """chunk argmax + winning-chunk regather (needs cm[t])."""
                nc.vector.max(out=top8[:, t, :], in_=cm[:, t, :])
                nc.vector.max_index(
                    out=cidx8s[:, t, :], in_max=top8[:, t, :], in_values=cm[:, t, :]
                )
                nc.gpsimd.tensor_tensor(
                    offs[:, t : t + 1], iotas[:, t : t + 1], cidx8s[:, t, 0:1],
                    op=mybir.AluOpType.add,
                )
                nc.gpsimd.indirect_dma_start(
                    out=gath[:, t, :],
                    out_offset=None,
                    in_=predflat,
                    in_offset=bass.IndirectOffsetOnAxis(ap=offs[:, t : t + 1], axis=0),
                )

            def stage_b(t):
                """within-chunk argmax + centers gather (needs gath[t])."""
                widx8 = sp.tile([P, 8], u32, tag="widx8")
                nc.vector.max_index(
                    out=widx8[:], in_max=top8[:, t, :], in_values=gath[:, t, :]
                )
                # label = cidx * CW + widx, computed on the Act engine as
                # Identity(cidx * scale + bias): keeps both muls/adds off the
                # throughput-critical DVE; values <= 6624 are exact in fp32
                nc.scalar.activation(
                    labt[:, t : t + 1], cidx8s[:, t, 0:1],
                    mybir.ActivationFunctionType.Identity,
                    bias=widx8[:, 0:1], scale=float(CW),
                )
                nc.gpsimd.indirect_dma_start(
                    out=ctile[:, t, :],
                    out_offset=None,
                    in_=cent[:],
                    in_offset=bass.IndirectOffsetOnAxis(ap=labt[:, t : t + 1], axis=0),
                )

            def stage_c(t):
                """distance + outputs (needs ctile[t])."""
                if t == LT:
                    # DVE-local distance: avoids GpSimd/Act hops in the tail
                    nc.vector.tensor_tensor(
                        diff[:, t, :], ftile[:, t, :], ctile[:, t, :],
                        op=mybir.AluOpType.subtract,
                    )
                    nc.vector.tensor_tensor(
                        sq[:, t, :], diff[:, t, :], diff[:, t, :],
                        op=mybir.AluOpType.mult,
                    )
                    nc.vector.tensor_reduce(
                        d2[:, t : t + 1], sq[:, t, :],
                        axis=mybir.AxisListType.X, op=mybir.AluOpType.add,
                    )
                else:
                    nc.gpsimd.tensor_tensor(
                        diff[:, t, :], ftile[:, t, :], ctile[:, t, :],
                        op=mybir.AluOpType.subtract,
                    )
                    nc.scalar.activation(
                        sq[:, t, :], diff[:, t, :],
                        mybir.ActivationFunctionType.Square,
                        accum_out=d2[:, t : t + 1],
                    )
                nc.gpsimd.dma_start(labs[:, t : t + 1], labt[:, t : t + 1])
                nc.gpsimd.dma_start(dist[:, t : t + 1], d2[:, t : t + 1])

            for t in range(NT):
                pt = pts[t]
                if t < LT:
                    nc.vector.reduce_max(
                        cm[:, t, :],
                        pt[:].rearrange("p (k q) -> p k q", q=CW),
                        axis=mybir.AxisListType.X,
                    )
                else:
                    for lo, hi, _ in LT_BOUNDS:
                        nc.vector.reduce_max(
                            cm[:, t, lo:hi],
                            pt[:, lo * CW : hi * CW].rearrange(
                                "p (k q) -> p k q", q=CW
                            ),
                            axis=mybir.AxisListType.X,
                        )
                # staged software pipeline: A(t) | B(t-1) | C(t-2) — each
                # stage's DMA round trip completes while later tiles stream
                stage_a(t)
                if t >= 1:
                    stage_b(t - 1)
                if t >= 2:
                    stage_c(t - 2)

            # epilogue: B(LT) first so the last tile's centers gather is
            # never head-of-line blocked behind C(LT-1) work
            stage_b(LT)
            stage_c(LT - 1)
            stage_c(LT)
    nc.compile()
    return nc


def kernel(features, predicts, centers):
    global LAST_RESULTS
    feats = np.ascontiguousarray(np.asarray(features).reshape(N, D), dtype=np.float32)
    preds = np.ascontiguousarray(np.asarray(predicts).reshape(N, C), dtype=np.float32)
    cents = np.ascontiguousarray(np.asarray(centers), dtype=np.float32)

    nc = _build()
    in_maps = []
    for i in range(NCORES):
        fshard = feats[i * NS : (i + 1) * NS]  # [1024, 96]
        # [P, NT*D] partition-major layout: row t*128+p -> [p, t*D:(t+1)*D]
        fT = np.ascontiguousarray(
            fshard.reshape(NT, P, D).transpose(1, 0, 2).reshape(P, NT * D)
        )
        in_maps.append(
            {
                "predicts": preds[i * NS : (i + 1) * NS],
                "features": fT,
                "centers": cents,
                "iotas": _IOTAS,
            }
        )
    res = run_bass_kernel_spmd(
        nc, in_maps, core_ids=list(range(NCORES)), trace=TRACE, **TRACE_KWARGS
    )
    LAST_RESULTS = res

    total = 0.0
    for r in res.results:
        # EPS clip of the per-row distances happens here as part of the
        # unshard-reduce (d2 >= 0 always; only the lower clip can bind)
        total += float(np.maximum(r["dists"], EPS).astype(np.float64).sum())
    total += float(N) * (C - 1) * EPS
    return np.asarray(total / N, dtype=np.float32)
